# revision 1
# baseline (speedup 1.0000x reference)
"""Causal GQA attention (B=2, S=2048, HID=2048, H=16, KVH=4, D=128) on 8 TRN2 cores.

Sharding: core c -> batch c//4, kv-group c%4 (4 q-heads + 1 kv-head).
o_proj is row-split by head group; host sums the 4 partials per batch.

Device kernel (per core, bf16 matmuls / f32 accumulation):
  xT (host-pretransposed [HID, S]) -> qT/kT/vT projections -> RoPE (pair-permuted
  on host into wq/wk columns, applied via half-swap + cos/sin muls) ->
  scores^T = K.Q^T per [128k, 512q] tile -> exp on ScalarE -> causal tri-mask on
  the diagonal 128-block -> AV^T with V-natural as stationary operand ->
  ones-matmul denominators -> normalize -> o_proj (out[t, o] += avN_h.T @ wo_h).
Diagonal supertiles only compute the valid q-range [128r:512].
"""

import numpy as np
import ml_dtypes

BF16 = ml_dtypes.bfloat16

B, S, HID = 2, 2048, 2048
H, KVH, D = 16, 4, 128
P = 128
KO = HID // P          # 16 contraction tiles
HQ = H // KVH          # 4 q heads per core
NTB = S // 512         # 4 token tiles of 512 (projection)
NQB = S // 512         # 4 query blocks of 512 (attention)
QW = 512
NKB = S // P           # 16 key blocks of 128
N_CORES = 8

_CACHE = {}


def _build_nc():
    import concourse.tile as tile
    from concourse import bacc, mybir
    from concourse.masks import make_identity
    from contextlib import ExitStack

    bf = mybir.dt.bfloat16
    f32 = mybir.dt.float32
    AF = mybir.ActivationFunctionType
    QS = 2 * QW  # 1024-wide scores/exp supertile

    nc = bacc.Bacc("TRN2", target_bir_lowering=False, debug=False,
                   num_devices=N_CORES)

    xT_d = nc.dram_tensor("xT", [HID, S], bf, kind="ExternalInput").ap()
    wq_d = nc.dram_tensor("wq", [P, KO * HQ * D], bf, kind="ExternalInput").ap()
    wk_d = nc.dram_tensor("wk", [P, KO * D], bf, kind="ExternalInput").ap()
    wv_d = nc.dram_tensor("wv", [P, KO * D], bf, kind="ExternalInput").ap()
    wo_d = nc.dram_tensor("wo", [P, HQ * HID], bf, kind="ExternalInput").ap()
    cs_d = nc.dram_tensor("cs2", [P, S], bf, kind="ExternalInput").ap()
    ss_d = nc.dram_tensor("ss2", [P, S], bf, kind="ExternalInput").ap()
    mk_d = nc.dram_tensor("mask", [P, P], bf, kind="ExternalInput").ap()
    out_d = nc.dram_tensor("out", [S, HID], bf, kind="ExternalOutput").ap()
    out_r = out_d.rearrange("(tb p) o -> p tb o", p=P)

    import concourse.bass_isa as bass_isa

    with tile.TileContext(nc) as tc:
        with ExitStack() as octx:
            const = octx.enter_context(tc.tile_pool(name="const", bufs=1))
            rope_p = octx.enter_context(tc.tile_pool(name="rope", bufs=3))
            at_p = octx.enter_context(tc.tile_pool(name="at", bufs=6))
            small = octx.enter_context(tc.tile_pool(name="small", bufs=4))
            xt_ctx = ExitStack()
            xt_pool = xt_ctx.enter_context(tc.tile_pool(name="xt", bufs=1))

            # ---- persistent loads (order matters: v/k weights before xT) ----
            wk_sb = const.tile([P, KO, D], bf, tag="wk", name="wk")
            nc.sync.dma_start(wk_sb[:], wk_d.rearrange("p (ko n) -> p ko n", ko=KO))
            wv_sb = const.tile([P, KO, D], bf, tag="wv", name="wv")
            nc.sync.dma_start(wv_sb[:], wv_d.rearrange("p (ko n) -> p ko n", ko=KO))
            xTk = []
            for ko in range(KO):
                t = xt_pool.tile([P, S], bf, tag=f"xT{ko}", name=f"xT{ko}")
                nc.sync.dma_start(t[:], xT_d[ko * P:(ko + 1) * P, :])
                xTk.append(t)
            cs_sb = const.tile([P, S], bf, tag="cs", name="cs")
            nc.sync.dma_start(cs_sb[:], cs_d[:])
            ss_sb = const.tile([P, S], bf, tag="ss", name="ss")
            nc.sync.dma_start(ss_sb[:], ss_d[:])
            wq_sb = const.tile([P, KO, HQ * D], bf, tag="wq", name="wq")
            nc.sync.dma_start(wq_sb[:], wq_d.rearrange("p (ko n) -> p ko n", ko=KO))
            mk_sb = const.tile([P, P], bf, tag="mk", name="mk")
            nc.sync.dma_start(mk_sb[:], mk_d[:])
            wo_sb = const.tile([P, HQ, HID], bf, tag="wo", name="wo")
            nc.sync.dma_start(wo_sb[:], wo_d.rearrange("p (h o) -> p h o", h=HQ))
            ident = const.tile([P, P], bf, tag="ident", name="ident")
            make_identity(nc, ident[:])
            ones = const.tile([P, 1], bf, tag="ones", name="ones")
            nc.vector.memset(ones[:], 1.0)

            qR = [const.tile([P, S], bf, tag=f"qR{h}", name=f"qR{h}")
                  for h in range(HQ)]
            kR = const.tile([P, S], bf, tag="kR", name="kR")
            vT_sb = const.tile([P, S], bf, tag="vT", name="vT")
            vN = const.tile([P, NKB, D], bf, tag="vN", name="vN")
            avN = [const.tile([P, S], bf, tag=f"avN{h}", name=f"avN{h}")
                   for h in range(HQ)]

            with ExitStack() as ctx:
                # PSUM: s = [P,1024]f32 (2 banks) x2, av x2, dn x2 -> 8 banks
                ps_s_p = ctx.enter_context(
                    tc.tile_pool(name="ps_s", bufs=2, space="PSUM"))
                ps_av_p = ctx.enter_context(
                    tc.tile_pool(name="ps_av", bufs=2, space="PSUM"))
                ps_dn_p = ctx.enter_context(
                    tc.tile_pool(name="ps_dn", bufs=2, space="PSUM"))

                def s_tile():
                    return ps_s_p.tile([P, QS], f32, tag="s", name="s")

                def proj_koouter(w3, col0):
                    """All four 512-token tiles at once (ko-outer) so PE work
                    starts as soon as the first xT chunk lands."""
                    ta, tb_ = s_tile(), s_tile()
                    quarters = [ta[:, 0:QW], ta[:, QW:QS],
                                tb_[:, 0:QW], tb_[:, QW:QS]]
                    for ko in range(KO):
                        for i in range(4):
                            nc.tensor.matmul(
                                quarters[i][:D, :],
                                lhsT=w3[:, ko, col0:col0 + D],
                                rhs=xTk[ko][:, i * QW:(i + 1) * QW],
                                start=(ko == 0), stop=(ko == KO - 1))
                    return quarters

                def rope_tile(ps, out_sl, tb):
                    tsl = slice(tb * QW, (tb + 1) * QW)
                    raw = rope_p.tile([P, QW], bf, tag="rp_raw", name="rp_raw")
                    nc.vector.tensor_copy(raw[:], ps[:])
                    sw = rope_p.tile([P, QW], bf, tag="rp_sw", name="rp_sw")
                    nc.sync.dma_start(sw[0:64, :], raw[64:128, :])
                    nc.sync.dma_start(sw[64:128, :], raw[0:64, :])
                    t1 = rope_p.tile([P, QW], bf, tag="rp_t1", name="rp_t1")
                    nc.vector.tensor_mul(t1[:], raw[:], cs_sb[:, tsl])
                    t2 = rope_p.tile([P, QW], bf, tag="rp_t2", name="rp_t2")
                    nc.vector.tensor_mul(t2[:], sw[:], ss_sb[:, tsl])
                    nc.vector.tensor_add(out_sl, t1[:], t2[:])

                # ---- HAM warmup: keep PE busy ~3.4us so the clock gate
                # opens before the DMA-paced projections start ----
                junk = ps_dn_p.tile([1, QW], f32, tag="dn", name="dn")
                for _ in range(32):
                    nc.tensor.matmul(junk[0:1, 0:P], lhsT=ones[:],
                                     rhs=ident[:], start=True, stop=True)

                # ---- v projection (ko-outer) ----
                vt = proj_koouter(wv_sb, 0)
                for tb in range(NTB):
                    nc.scalar.copy(vT_sb[:, tb * QW:(tb + 1) * QW], vt[tb][:])
                # ---- k projection (ko-outer) + rope ----
                kt = proj_koouter(wk_sb, 0)
                for tb in range(NTB):
                    rope_tile(kt[tb], kR[:, tb * QW:(tb + 1) * QW], tb)
                # ---- v transpose to natural layout ----
                for kb in range(NKB):
                    pst = ps_av_p.tile([P, QW], bf, tag="av", name="av")
                    nc.tensor.transpose(
                        pst[:, 0:P], vT_sb[:, kb * P:(kb + 1) * P], ident[:])
                    nc.scalar.copy(vN[:, kb, :], pst[:, 0:P])
                # ---- q projections ----
                for h in range(HQ):
                    for pair in range(2):
                        t = s_tile()
                        halves = [t[:, 0:QW], t[:, QW:QS]]
                        for ko in range(KO):
                            for i in range(2):
                                tb = 2 * pair + i
                                nc.tensor.matmul(
                                    halves[i][:D, :],
                                    lhsT=wq_sb[:, ko, h * D:(h + 1) * D],
                                    rhs=xTk[ko][:, tb * QW:(tb + 1) * QW],
                                    start=(ko == 0), stop=(ko == KO - 1))
                        for i in range(2):
                            tb = 2 * pair + i
                            rope_tile(halves[i],
                                      qR[h][:, tb * QW:(tb + 1) * QW], tb)

                xt_ctx.close()  # xT tiles dead; frees SBUF

                seq = [(h, j, kb)
                       for h in range(HQ)
                       for j in range(2)
                       for kb in range(8 * j + 8)]
                st = {}

                def scores_i(h, j, kb):
                    nkb = 8 * j + 8
                    q0 = j * QS
                    if kb == 0:
                        st[(h, j)] = {
                            "av": [ps_av_p.tile([P, QW], f32, tag="av",
                                                name="av") for _ in range(2)],
                            "dn": [ps_dn_p.tile([1, QW], f32, tag="dn",
                                                name="dn") for _ in range(2)],
                            "ats": {},
                        }
                    s = st[(h, j)]
                    r = kb - 8 * j
                    lo = 128 * r if r >= 0 else 0
                    ps_s = s_tile()
                    if lo < QW:
                        nc.tensor.matmul(
                            ps_s[:, lo:QW],
                            lhsT=kR[:, kb * P:(kb + 1) * P],
                            rhs=qR[h][:, q0 + lo:q0 + QW],
                            start=True, stop=True)
                    l1 = max(lo, QW)
                    nc.tensor.matmul(
                        ps_s[:, l1:QS],
                        lhsT=kR[:, kb * P:(kb + 1) * P],
                        rhs=qR[h][:, q0 + l1:q0 + QS],
                        start=True, stop=True)
                    at = at_p.tile([P, QS], bf, tag="at", name="at")
                    nc.scalar.activation(at[:, lo:QS], ps_s[:, lo:QS], AF.Exp)
                    if r >= 0:
                        nc.vector.tensor_mul(
                            at[:, lo:lo + P], at[:, lo:lo + P], mk_sb[:])
                    s["ats"][kb] = at

                def accum_i(h, j, kb):
                    nkb = 8 * j + 8
                    q0 = j * QS
                    s = st[(h, j)]
                    ps_av, ps_dn = s["av"], s["dn"]
                    r = kb - 8 * j
                    lo = 128 * r if r >= 0 else 0
                    at = s["ats"].pop(kb)
                    l1 = max(lo, QW)
                    half0 = kb <= 8 * j + 3
                    if half0:  # av pair shares lhsT=vN[kb]
                        nc.tensor.matmul(
                            ps_av[0][:, lo:QW], lhsT=vN[:, kb, :],
                            rhs=at[:, lo:QW],
                            start=(kb == 0), stop=(kb == 8 * j + 3))
                    nc.tensor.matmul(
                        ps_av[1][:, l1 - QW:QW], lhsT=vN[:, kb, :],
                        rhs=at[:, l1:QS],
                        start=(kb == 0), stop=(kb == nkb - 1))
                    if half0:  # dn pair shares lhsT=ones
                        nc.tensor.matmul(
                            ps_dn[0][0:1, lo:QW], lhsT=ones[:],
                            rhs=at[:, lo:QW],
                            start=(kb == 0), stop=(kb == 8 * j + 3))
                    nc.tensor.matmul(
                        ps_dn[1][0:1, l1 - QW:QW], lhsT=ones[:],
                        rhs=at[:, l1:QS],
                        start=(kb == 0), stop=(kb == nkb - 1))
                    if kb == nkb - 1:
                        # free the psum banks fast: stage unnormalized AV to
                        # SBUF with one ScalarE copy per half, normalize later
                        for half in range(2):
                            avu = small.tile([P, QW], f32, tag="avu",
                                             name="avu")
                            nc.vector.tensor_copy(avu[:], ps_av[half][:])
                            recip = small.tile([1, QW], f32, tag="recip",
                                               name="recip")
                            nc.vector.reciprocal_approx_fast(
                                recip[:], ps_dn[half][:])
                            rb = small.tile([P, QW], f32, tag="rb", name="rb")
                            nc.gpsimd.partition_broadcast(rb[:], recip[:])
                            nc.vector.tensor_mul(
                                avN[h][:, q0 + half * QW:q0 + (half + 1) * QW],
                                avu[:], rb[:])
                        del st[(h, j)]

                scores_i(*seq[0])
                for i in range(1, len(seq)):
                    scores_i(*seq[i])
                    accum_i(*seq[i - 1])
                accum_i(*seq[-1])


            # ---- o_proj ----
            with ExitStack() as ctx:
                op_p = ctx.enter_context(
                    tc.tile_pool(name="op", bufs=2, space="PSUM"))
                ost_p = ctx.enter_context(tc.tile_pool(name="ost", bufs=4))
                for tb in range(NKB):  # 16 token tiles of 128
                    pso = op_p.tile([P, 4, QW], f32, tag="o", name="o")
                    for hh in range(HQ):
                        for ob in range(4):
                            nc.tensor.matmul(
                                pso[:, ob, :],
                                lhsT=avN[hh][:, tb * P:(tb + 1) * P],
                                rhs=wo_sb[:, hh, ob * QW:(ob + 1) * QW],
                                start=(hh == 0), stop=(hh == HQ - 1))
                    psof = pso.rearrange("p a b -> p (a b)")
                    for hf in range(2):
                        ot = ost_p.tile([P, 2 * QW], bf, tag="ot", name="ot")
                        if hf == 0:
                            nc.scalar.copy(
                                ot[:], psof[:, hf * 2 * QW:(hf + 1) * 2 * QW])
                        else:
                            nc.vector.tensor_copy(
                                ot[:], psof[:, hf * 2 * QW:(hf + 1) * 2 * QW])
                        nc.sync.dma_start(
                            out_r[:, tb, hf * 2 * QW:(hf + 1) * 2 * QW], ot[:])

    nc.compile()
    return nc


def _prep_inputs(x, freqs_cis, wq, wk, wv, wo):
    x = np.asarray(x, dtype=np.float32)
    freqs = np.asarray(freqs_cis, dtype=np.float32)
    wq = np.asarray(wq, dtype=np.float32)
    wk = np.asarray(wk, dtype=np.float32)
    wv = np.asarray(wv, dtype=np.float32)
    wo = np.asarray(wo, dtype=np.float32)

    perm = np.concatenate([np.arange(0, D, 2), np.arange(1, D, 2)])
    cos = freqs[..., 0].T.astype(np.float32)            # [64, S]
    sin = freqs[..., 1].T.astype(np.float32)
    cs2 = np.ascontiguousarray(np.concatenate([cos, cos], 0)).astype(BF16)
    ss2 = np.ascontiguousarray(np.concatenate([-sin, sin], 0)).astype(BF16)

    wq_p = (wq.reshape(HID, H, D)[:, :, perm] * D**-0.5).astype(BF16)
    wk_p = wk.reshape(HID, KVH, D)[:, :, perm].astype(BF16)
    wv_r = wv.reshape(HID, KVH, D).astype(BF16)
    wo_r = wo.reshape(H, D, HID)

    kk = np.arange(P)[:, None]
    qq = np.arange(P)[None, :]
    tri = (kk <= qq).astype(BF16)                        # [128, 128]

    xT = np.ascontiguousarray(x.transpose(0, 2, 1)).astype(BF16)  # [B, HID, S]

    def swz(w):  # [HID, N] -> [P, KO*N] so each partition's DMA is contiguous
        n = w.shape[1]
        return np.ascontiguousarray(
            w.reshape(KO, P, n).transpose(1, 0, 2).reshape(P, KO * n))

    in_maps = []
    for c in range(N_CORES):
        b, g = c // 4, c % 4
        wo_g = wo_r[4 * g:4 * g + HQ].astype(BF16)      # [HQ, P, HID]
        in_maps.append({
            "xT": xT[b],
            "wq": swz(wq_p[:, 4 * g:4 * g + HQ, :].reshape(HID, HQ * D)),
            "wk": swz(wk_p[:, g, :]),
            "wv": swz(wv_r[:, g, :]),
            "wo": np.ascontiguousarray(
                wo_g.transpose(1, 0, 2).reshape(P, HQ * HID)),
            "cs2": cs2,
            "ss2": ss2,
            "mask": tri,
        })
    return in_maps


def _ensure_ntff_hook():
    """Optional: register the NTFF profiling hook if the image's antenv lacks
    it, so BASS_TRACE=1 produces a profile instead of crashing. No-op on
    failure or when the hook already exists."""
    import sys as _sys
    import types as _types
    try:
        from antenv.axon_hooks import get_axon_ntff_profile_hook  # noqa: F401
        return
    except ImportError:
        pass
    try:
        from trn_agent_boot.trn_boot import _ntff_profile_via_ctypes
        hook = _ntff_profile_via_ctypes("/opt/axon/libaxon_pjrt.so")
        mod = _types.ModuleType("antenv.axon_hooks")
        mod.get_axon_ntff_profile_hook = lambda: hook
        mod.set_axon_ntff_profile_hook = lambda h: None
        _sys.modules["antenv.axon_hooks"] = mod
    except Exception:
        pass


def kernel(x, freqs_cis, wq, wk, wv, wo):
    from concourse.bass_utils import run_bass_kernel_spmd
    _ensure_ntff_hook()

    nc = _CACHE.get("nc")
    if nc is None:
        nc = _build_nc()
        _CACHE["nc"] = nc

    in_maps = _prep_inputs(x, freqs_cis, wq, wk, wv, wo)
    res = run_bass_kernel_spmd(nc, in_maps, list(range(N_CORES)))
    _CACHE["last_result"] = res
    parts = [np.asarray(res.results[c]["out"]).astype(np.float32)
             for c in range(N_CORES)]
    out = np.stack([parts[0] + parts[1] + parts[2] + parts[3],
                    parts[4] + parts[5] + parts[6] + parts[7]])
    return out



# revision 7
# speedup vs baseline: 1.0220x; 1.0220x over previous
"""Causal GQA attention (B=2, S=2048, HID=2048, H=16, KVH=4, D=128) on 8 TRN2 cores.

Sharding: core c -> batch c//4, kv-group c%4 (4 q-heads + 1 kv-head).
o_proj is row-split by head group; host sums the 4 partials per batch.

Device kernel (per core, fp16 matmuls / f32 accumulation), fully streamed:
the sequence is processed in four 512-token quarters. Per quarter: v/k
projections (ko-interleaved so PE starts as soon as each xT chunk lands),
q projections, then attention for that query quarter (kv blocks 0..end of
quarter) and o_proj for the previous quarter are interleaved between later
projections so the PE never sits behind a DMA or phase barrier.

Softmax denominators are NOT computed with ones-matmuls on the PE (that
doubles attention PE traffic); instead the exp tiles are accumulated
elementwise on the DVE (fp16 atsum) and a single [1,512] ones-matmul per
(head, quarter) reduces across partitions.
"""

import numpy as np
import ml_dtypes

F16 = np.float16

B, S, HID = 2, 2048, 2048
H, KVH, D = 16, 4, 128
P = 128
KO = HID // P          # 16 contraction chunks of 128
HQ = H // KVH          # 4 q heads per core
NQ = 4                 # token quarters
QT = S // NQ           # 512 tokens per quarter
NKB = S // P           # 16 key blocks of 128
N_CORES = 8

_CACHE = {}


def _build_nc():
    import concourse.tile as tile
    from concourse import bacc, mybir
    from concourse.masks import make_identity
    from contextlib import ExitStack

    f16 = mybir.dt.float16
    f32 = mybir.dt.float32
    AF = mybir.ActivationFunctionType

    nc = bacc.Bacc("TRN2", target_bir_lowering=False, debug=False,
                   num_devices=N_CORES)

    xT_d = nc.dram_tensor("xT", [NQ * KO * P, QT], f16, kind="ExternalInput").ap()
    wq_d = nc.dram_tensor("wq", [P, HQ * KO * D], f16, kind="ExternalInput").ap()
    wk_d = nc.dram_tensor("wk", [P, KO * D], f16, kind="ExternalInput").ap()
    wv_d = nc.dram_tensor("wv", [P, KO * D], f16, kind="ExternalInput").ap()
    wo_d = nc.dram_tensor("wo", [P, HQ * HID], f16, kind="ExternalInput").ap()
    cs_d = nc.dram_tensor("cs2", [P, S], f16, kind="ExternalInput").ap()
    ss_d = nc.dram_tensor("ss2", [P, S], f16, kind="ExternalInput").ap()
    mk_d = nc.dram_tensor("mask", [P, P], f16, kind="ExternalInput").ap()
    out_d = nc.dram_tensor("out", [S, HID], f16, kind="ExternalOutput").ap()
    out_r = out_d.rearrange("(tb p) o -> p tb o", p=P)

    with tile.TileContext(nc) as tc:
        with ExitStack() as octx:
            const = octx.enter_context(tc.tile_pool(name="const", bufs=1))
            rope_p = octx.enter_context(tc.tile_pool(name="rope", bufs=3))
            at_p = octx.enter_context(tc.tile_pool(name="at", bufs=5))
            asum_p = octx.enter_context(tc.tile_pool(name="asum", bufs=2))
            small = octx.enter_context(tc.tile_pool(name="small", bufs=3))
            ost_p = octx.enter_context(tc.tile_pool(name="ost", bufs=4))
            xt_ctx = ExitStack()
            xt_pool = xt_ctx.enter_context(tc.tile_pool(name="xt", bufs=1))

            # ---- persistent input DMAs, emitted in consumption order ----
            wk_sb = const.tile([P, KO, D], f16, tag="wk", name="wk")
            nc.sync.dma_start(wk_sb[:], wk_d.rearrange("p (ko n) -> p ko n", ko=KO))
            wv_sb = const.tile([P, KO, D], f16, tag="wv", name="wv")
            nc.sync.dma_start(wv_sb[:], wv_d.rearrange("p (ko n) -> p ko n", ko=KO))

            wq_sb = const.tile([P, HQ, KO, D], f16, tag="wq", name="wq")
            wq_r = wq_d.rearrange("p (h ko n) -> p h ko n", h=HQ, ko=KO)
            nc.sync.dma_start(wq_sb[:, 0], wq_r[:, 0])

            xTk = [[None] * KO for _ in range(NQ)]
            for ko in range(KO):
                t = xt_pool.tile([P, QT], f16, tag=f"xT0_{ko}", name=f"xT0_{ko}")
                r0 = (0 * KO + ko) * P
                nc.sync.dma_start(t[:], xT_d[r0:r0 + P, :])
                xTk[0][ko] = t

            for h in range(1, HQ):
                nc.sync.dma_start(wq_sb[:, h], wq_r[:, h])

            cs_sb = const.tile([P, S], f16, tag="cs", name="cs")
            nc.sync.dma_start(cs_sb[:], cs_d[:])
            ss_sb = const.tile([P, S], f16, tag="ss", name="ss")
            nc.sync.dma_start(ss_sb[:], ss_d[:])
            mk_sb = const.tile([P, P], f16, tag="mk", name="mk")
            nc.sync.dma_start(mk_sb[:], mk_d[:])

            for tq in range(1, NQ):
                for ko in range(KO):
                    t = xt_pool.tile([P, QT], f16, tag=f"xT{tq}_{ko}",
                                     name=f"xT{tq}_{ko}")
                    r0 = (tq * KO + ko) * P
                    nc.sync.dma_start(t[:], xT_d[r0:r0 + P, :])
                    xTk[tq][ko] = t

            wo_sb = const.tile([P, HQ, HID], f16, tag="wo", name="wo")
            nc.sync.dma_start(wo_sb[:], wo_d.rearrange("p (h o) -> p h o", h=HQ))

            ident = const.tile([P, P], f16, tag="ident", name="ident")
            make_identity(nc, ident[:])
            ones = const.tile([P, 1], f16, tag="ones", name="ones")
            nc.vector.memset(ones[:], 1.0)

            qR = [const.tile([P, S], f16, tag=f"qR{h}", name=f"qR{h}")
                  for h in range(HQ)]
            kR = const.tile([P, S], f16, tag="kR", name="kR")
            vN = const.tile([P, NKB, D], f16, tag="vN", name="vN")
            avN = [const.tile([P, S], f16, tag=f"avN{h}", name=f"avN{h}")
                   for h in range(HQ)]

            with ExitStack() as ctx:
                # PSUM: proj(2) + s(2) + av(2) + oproj(2) = 8 banks
                proj_pp = ctx.enter_context(
                    tc.tile_pool(name="ps_proj", bufs=2, space="PSUM"))
                s_pp = ctx.enter_context(
                    tc.tile_pool(name="ps_s", bufs=2, space="PSUM"))
                av_pp = ctx.enter_context(
                    tc.tile_pool(name="ps_av", bufs=2, space="PSUM"))
                o_pp = ctx.enter_context(
                    tc.tile_pool(name="ps_o", bufs=2, space="PSUM"))

                def rope_tile(ps, out_sl, tsl):
                    raw = rope_p.tile([P, QT], f16, tag="rp_raw", name="rp_raw")
                    nc.scalar.copy(raw[:], ps[:])
                    sw = rope_p.tile([P, QT], f16, tag="rp_sw", name="rp_sw")
                    nc.sync.dma_start(sw[0:64, :], raw[64:128, :])
                    nc.sync.dma_start(sw[64:128, :], raw[0:64, :])
                    t1 = rope_p.tile([P, QT], f16, tag="rp_t1", name="rp_t1")
                    nc.vector.tensor_mul(t1[:], raw[:], cs_sb[:, tsl])
                    t2 = rope_p.tile([P, QT], f16, tag="rp_t2", name="rp_t2")
                    nc.vector.tensor_mul(t2[:], sw[:], ss_sb[:, tsl])
                    nc.vector.tensor_add(out_sl, t1[:], t2[:])

                def proj_kq0(tq):
                    """k and q-head-0 projections, ko-interleaved so the PE
                    starts on each xT chunk as soon as its DMA lands. Each
                    GEMM accumulates in its own PSUM bank — hardware PSUM
                    accumulation breaks if two start=True groups share a
                    bank."""
                    k_ps = proj_pp.tile([P, QT], f32, tag="proj", name="k_ps")
                    q_ps = proj_pp.tile([P, QT], f32, tag="proj", name="q_ps")
                    for ko in range(KO):
                        st, sp = ko == 0, ko == KO - 1
                        x = xTk[tq][ko]
                        nc.tensor.matmul(k_ps[:], lhsT=wk_sb[:, ko, :],
                                         rhs=x[:], start=st, stop=sp)
                        nc.tensor.matmul(q_ps[:], lhsT=wq_sb[:, 0, ko, :],
                                         rhs=x[:], start=st, stop=sp)
                    tsl = slice(tq * QT, (tq + 1) * QT)
                    rope_tile(k_ps, kR[:, tsl], tsl)
                    rope_tile(q_ps, qR[0][:, tsl], tsl)

                def proj_v(tq):
                    """v projection straight to natural [tok, d] layout:
                    lhsT = xT chunk block, rhs = wv. The four 128-token
                    blocks run sequentially, each in its own psum ring slot
                    (one open accumulation group per bank)."""
                    for i in range(4):
                        v_ps = proj_pp.tile([P, P], f32, tag="proj",
                                            name="v_ps")
                        for ko in range(KO):
                            nc.tensor.matmul(
                                v_ps[:],
                                lhsT=xTk[tq][ko][:, i * P:(i + 1) * P],
                                rhs=wv_sb[:, ko, :],
                                start=(ko == 0), stop=(ko == KO - 1))
                        nc.scalar.copy(vN[:, 4 * tq + i, :], v_ps[:])

                def proj_q(tq, h):
                    q_ps = proj_pp.tile([P, QT], f32, tag="proj", name="q_ps")
                    for ko in range(KO):
                        nc.tensor.matmul(
                            q_ps[:], lhsT=wq_sb[:, h, ko, :],
                            rhs=xTk[tq][ko][:],
                            start=(ko == 0), stop=(ko == KO - 1))
                    tsl = slice(tq * QT, (tq + 1) * QT)
                    rope_tile(q_ps, qR[h][:, tsl], tsl)

                # ---- attention for query quarter tq, head h ----
                # per-kb: scores -> exp -> (tri-mask) -> {AV matmul, atsum}
                # issue scores[i+1] before AV[i] so the PE always has work
                # while the ScalarE exps the previous block.
                ast = {}

                def attn_scores(tq, h, kb):
                    nkb = 4 * tq + 4
                    q0 = tq * QT
                    if kb == 0:
                        ast[(tq, h)] = {
                            "av": av_pp.tile([P, QT], f32, tag="av", name="av"),
                            "asum": asum_p.tile([P, QT], f16, tag="asum",
                                                name="asum"),
                            "ats": {},
                        }
                    st = ast[(tq, h)]
                    r = kb - 4 * tq
                    lo = P * r if r >= 0 else 0
                    s_ps = s_pp.tile([P, QT], f32, tag="s", name="s")
                    nc.tensor.matmul(
                        s_ps[:, lo:QT],
                        lhsT=kR[:, kb * P:(kb + 1) * P],
                        rhs=qR[h][:, q0 + lo:q0 + QT],
                        start=True, stop=True)
                    at = at_p.tile([P, QT], f16, tag="at", name="at")
                    nc.scalar.activation(at[:, lo:QT], s_ps[:, lo:QT], AF.Exp)
                    if r >= 0:
                        nc.vector.tensor_mul(
                            at[:, lo:lo + P], at[:, lo:lo + P], mk_sb[:])
                    if kb == 0:
                        nc.vector.tensor_copy(st["asum"][:], at[:])
                    else:
                        nc.vector.tensor_add(st["asum"][:, lo:QT],
                                             st["asum"][:, lo:QT], at[:, lo:QT])
                    st["ats"][kb] = at

                def attn_av(tq, h, kb):
                    nkb = 4 * tq + 4
                    q0 = tq * QT
                    st = ast[(tq, h)]
                    r = kb - 4 * tq
                    lo = P * r if r >= 0 else 0
                    at = st["ats"].pop(kb)
                    nc.tensor.matmul(
                        st["av"][:, lo:QT], lhsT=vN[:, kb, :],
                        rhs=at[:, lo:QT],
                        start=(kb == 0), stop=(kb == nkb - 1))
                    if kb == nkb - 1:
                        dn_ps = s_pp.tile([1, QT], f32, tag="s", name="dn_ps")
                        nc.tensor.matmul(dn_ps[0:1, :], lhsT=ones[:],
                                         rhs=st["asum"][:], start=True,
                                         stop=True)
                        rc = small.tile([1, QT], f32, tag="rc", name="rc")
                        nc.vector.reciprocal_approx_fast(rc[:], dn_ps[:])
                        rb = small.tile([P, QT], f32, tag="rb", name="rb")
                        nc.gpsimd.partition_broadcast(rb[:], rc[:])
                        nc.vector.tensor_mul(
                            avN[h][:, q0:q0 + QT], st["av"][:], rb[:])
                        del ast[(tq, h)]

                def attn_quarter(tq):
                    seq = [(h, kb) for h in range(HQ)
                           for kb in range(4 * tq + 4)]
                    attn_scores(tq, seq[0][0], seq[0][1])
                    for i in range(1, len(seq)):
                        attn_scores(tq, *seq[i])
                        attn_av(tq, *seq[i - 1])
                    attn_av(tq, *seq[-1])

                def oproj_quarter(tq):
                    for tb in range(4 * tq, 4 * tq + 4):
                        for ob in range(4):
                            o_ps = o_pp.tile([P, QT], f32, tag="o", name="o_ps")
                            for hh in range(HQ):
                                nc.tensor.matmul(
                                    o_ps[:],
                                    lhsT=avN[hh][:, tb * P:(tb + 1) * P],
                                    rhs=wo_sb[:, hh, ob * QT:(ob + 1) * QT],
                                    start=(hh == 0), stop=(hh == HQ - 1))
                            ot = ost_p.tile([P, QT], f16, tag="ot", name="ot")
                            if ob % 2 == 0:
                                nc.scalar.copy(ot[:], o_ps[:])
                            else:
                                nc.vector.tensor_copy(ot[:], o_ps[:])
                            nc.sync.dma_start(
                                out_r[:, tb, ob * QT:(ob + 1) * QT], ot[:])

                # ---- HAM warmup: keep the PE busy ~3.5us so the clock
                # ramps before the DMA-paced first projections ----
                junk = s_pp.tile([1, QT], f32, tag="s", name="junk")
                for _ in range(32):
                    nc.tensor.matmul(junk[0:1, 0:P], lhsT=ones[:],
                                     rhs=ident[:], start=True, stop=True)

                # ---- streamed schedule ----
                def proj_quarter(tq):
                    proj_kq0(tq)
                    proj_v(tq)
                    for h in range(1, HQ):
                        proj_q(tq, h)

                proj_quarter(0)
                proj_quarter(1)
                attn_quarter(0)
                proj_quarter(2)
                attn_quarter(1)
                oproj_quarter(0)
                proj_quarter(3)
                xt_ctx.close()
                attn_quarter(2)
                oproj_quarter(1)
                attn_quarter(3)
                oproj_quarter(2)
                oproj_quarter(3)

    nc.compile()
    return nc


def _prep_inputs(x, freqs_cis, wq, wk, wv, wo):
    x = np.asarray(x, dtype=np.float32)
    freqs = np.asarray(freqs_cis, dtype=np.float32)
    wq = np.asarray(wq, dtype=np.float32)
    wk = np.asarray(wk, dtype=np.float32)
    wv = np.asarray(wv, dtype=np.float32)
    wo = np.asarray(wo, dtype=np.float32)

    perm = np.concatenate([np.arange(0, D, 2), np.arange(1, D, 2)])
    cos = freqs[..., 0].T.astype(np.float32)            # [64, S]
    sin = freqs[..., 1].T.astype(np.float32)
    cs2 = np.ascontiguousarray(np.concatenate([cos, cos], 0)).astype(F16)
    ss2 = np.ascontiguousarray(np.concatenate([-sin, sin], 0)).astype(F16)

    wq_p = (wq.reshape(HID, H, D)[:, :, perm] * D**-0.5).astype(F16)
    wk_p = wk.reshape(HID, KVH, D)[:, :, perm].astype(F16)
    wv_r = wv.reshape(HID, KVH, D).astype(F16)
    wo_r = wo.reshape(H, D, HID)

    kk = np.arange(P)[:, None]
    qq = np.arange(P)[None, :]
    tri = (kk <= qq).astype(F16)                        # [128, 128]

    # xT chunks: [NQ, KO, P, QT] contiguous so each (quarter, ko) chunk is
    # one dense 128KB DMA
    xT = x.transpose(0, 2, 1).reshape(B, KO, P, NQ, QT)
    xT = np.ascontiguousarray(xT.transpose(0, 3, 1, 2, 4)).astype(F16)
    xT = xT.reshape(B, NQ * KO * P, QT)

    def swz(w):  # [HID, N] -> [P, KO*N] so each partition's DMA is contiguous
        n = w.shape[1]
        return np.ascontiguousarray(
            w.reshape(KO, P, n).transpose(1, 0, 2).reshape(P, KO * n))

    in_maps = []
    for c in range(N_CORES):
        b, g = c // 4, c % 4
        # wq host layout [P, HQ, KO, D]: per-head contiguous for split DMAs
        wq_g = wq_p[:, 4 * g:4 * g + HQ, :]             # [HID, HQ, D]
        wq_sw = wq_g.reshape(KO, P, HQ, D).transpose(1, 2, 0, 3)
        wq_sw = np.ascontiguousarray(wq_sw).reshape(P, HQ * KO * D)
        wo_g = wo_r[4 * g:4 * g + HQ].astype(F16)       # [HQ, P, HID]
        in_maps.append({
            "xT": xT[b],
            "wq": wq_sw,
            "wk": swz(wk_p[:, g, :]),
            "wv": swz(wv_r[:, g, :]),
            "wo": np.ascontiguousarray(
                wo_g.transpose(1, 0, 2).reshape(P, HQ * HID)),
            "cs2": cs2,
            "ss2": ss2,
            "mask": tri,
        })
    return in_maps


def _ensure_ntff_hook():
    """Optional: register the NTFF profiling hook if the image's antenv lacks
    it, so BASS_TRACE=1 produces a profile instead of crashing. No-op on
    failure or when the hook already exists."""
    import sys as _sys
    import types as _types
    try:
        from antenv.axon_hooks import get_axon_ntff_profile_hook  # noqa: F401
        return
    except ImportError:
        pass
    try:
        from trn_agent_boot.trn_boot import _ntff_profile_via_ctypes
        hook = _ntff_profile_via_ctypes("/opt/axon/libaxon_pjrt.so")
        mod = _types.ModuleType("antenv.axon_hooks")
        mod.get_axon_ntff_profile_hook = lambda: hook
        mod.set_axon_ntff_profile_hook = lambda h: None
        _sys.modules["antenv.axon_hooks"] = mod
    except Exception:
        pass


def kernel(x, freqs_cis, wq, wk, wv, wo):
    from concourse.bass_utils import run_bass_kernel_spmd
    _ensure_ntff_hook()

    nc = _CACHE.get("nc")
    if nc is None:
        nc = _build_nc()
        _CACHE["nc"] = nc

    in_maps = _prep_inputs(x, freqs_cis, wq, wk, wv, wo)
    res = run_bass_kernel_spmd(nc, in_maps, list(range(N_CORES)))
    _CACHE["last_result"] = res
    parts = [np.asarray(res.results[c]["out"]).astype(np.float32)
             for c in range(N_CORES)]
    out = np.stack([parts[0] + parts[1] + parts[2] + parts[3],
                    parts[4] + parts[5] + parts[6] + parts[7]])
    return out


# revision 13
# speedup vs baseline: 1.0294x; 1.0072x over previous
"""Causal GQA attention (B=2, S=2048, HID=2048, H=16, KVH=4, D=128) on 8 TRN2 cores.

Sharding: core c -> batch c//4, kv-group c%4 (4 q-heads + 1 kv-head).
o_proj is row-split by head group; host sums the 4 partials per batch.

Device kernel (per core, fp16 matmuls / f32 accumulation), fully streamed:
the sequence is processed in four 512-token quarters. Per quarter: v/k
projections (ko-interleaved so PE starts as soon as each xT chunk lands),
q projections, then attention for that query quarter (kv blocks 0..end of
quarter) and o_proj for the previous quarter are interleaved between later
projections so the PE never sits behind a DMA or phase barrier.

Softmax denominators are NOT computed with ones-matmuls on the PE (that
doubles attention PE traffic); instead the exp tiles are accumulated
elementwise on the DVE (fp16 atsum) and a single [1,512] ones-matmul per
(head, quarter) reduces across partitions.
"""

import numpy as np
import ml_dtypes

F16 = np.float16

B, S, HID = 2, 2048, 2048
H, KVH, D = 16, 4, 128
P = 128
KO = HID // P          # 16 contraction chunks of 128
HQ = H // KVH          # 4 q heads per core
NQ = 4                 # token quarters
QT = S // NQ           # 512 tokens per quarter
NKB = S // P           # 16 key blocks of 128
N_CORES = 8

_CACHE = {}


def _build_nc():
    import concourse.tile as tile
    from concourse import bacc, mybir
    from concourse.masks import make_identity
    from contextlib import ExitStack

    f16 = mybir.dt.float16
    f32 = mybir.dt.float32
    AF = mybir.ActivationFunctionType

    nc = bacc.Bacc("TRN2", target_bir_lowering=False, debug=False,
                   num_devices=N_CORES)

    xT_d = nc.dram_tensor("xT", [NQ * 4 * P, 4 * QT], f16, kind="ExternalInput").ap()
    wq_d = nc.dram_tensor("wq", [P, HQ * KO * D], f16, kind="ExternalInput").ap()
    wk_d = nc.dram_tensor("wk", [P, KO * D], f16, kind="ExternalInput").ap()
    wv_d = nc.dram_tensor("wv", [P, KO * D], f16, kind="ExternalInput").ap()
    wo_d = nc.dram_tensor("wo", [P, HQ * HID], f16, kind="ExternalInput").ap()
    cs_d = nc.dram_tensor("cs2", [P, S], f16, kind="ExternalInput").ap()
    ss_d = nc.dram_tensor("ss2", [P, S], f16, kind="ExternalInput").ap()
    mk_d = nc.dram_tensor("mask", [P, P], f16, kind="ExternalInput").ap()
    out_d = nc.dram_tensor("out", [S, HID], f16, kind="ExternalOutput").ap()
    out_r = out_d.rearrange("(tb p) o -> p tb o", p=P)

    with tile.TileContext(nc) as tc:
        with ExitStack() as octx:
            const = octx.enter_context(tc.tile_pool(name="const", bufs=1))
            rope_p = octx.enter_context(tc.tile_pool(name="rope", bufs=3))
            at_p = octx.enter_context(tc.tile_pool(name="at", bufs=5))
            asum_p = octx.enter_context(tc.tile_pool(name="asum", bufs=2))
            small = octx.enter_context(tc.tile_pool(name="small", bufs=3))
            ost_p = octx.enter_context(tc.tile_pool(name="ost", bufs=4))
            xt_ctx = ExitStack()
            xt_pool = xt_ctx.enter_context(tc.tile_pool(name="xt", bufs=1))

            # ---- persistent input DMAs, emitted in consumption order.
            # The DMA queues are descriptor-rate-bound (~4KB/partition per
            # descriptor is the sweet spot) and one dma_start lands on one
            # queue, so every big tensor is packed into 4KB-per-partition
            # runs and split into partition-halves to parallelize across
            # queues. ----
            def dma2(dst, src):
                nc.sync.dma_start(dst[0:64], src[0:64])
                nc.sync.dma_start(dst[64:128], src[64:128])

            wk_sb = const.tile([P, KO, D], f16, tag="wk", name="wk")
            dma2(wk_sb, wk_d.rearrange("p (ko n) -> p ko n", ko=KO))
            wv_sb = const.tile([P, KO, D], f16, tag="wv", name="wv")
            dma2(wv_sb, wv_d.rearrange("p (ko n) -> p ko n", ko=KO))

            wq_sb = const.tile([P, HQ, KO, D], f16, tag="wq", name="wq")
            wq_r = wq_d.rearrange("p (h ko n) -> p h ko n", h=HQ, ko=KO)
            dma2(wq_sb[:, 0], wq_r[:, 0])

            # xT arrives as groups of 4 ko-chunks (4KB/partition per DMA)
            xTg = [[None] * 4 for _ in range(NQ)]

            def load_xq(tq):
                for kg in range(4):
                    t = xt_pool.tile([P, 4, QT], f16, tag=f"xT{tq}_{kg}",
                                     name=f"xT{tq}_{kg}")
                    base = (tq * 4 + kg) * P
                    dma2(t.rearrange("p a b -> p (a b)"),
                         xT_d[base:base + P, :])
                    xTg[tq][kg] = t

            def xchunk(tq, ko):
                return xTg[tq][ko // 4][:, ko % 4, :]

            load_xq(0)

            for h in range(1, HQ):
                dma2(wq_sb[:, h], wq_r[:, h])

            cs_sb = const.tile([P, S], f16, tag="cs", name="cs")
            dma2(cs_sb, cs_d)
            ss_sb = const.tile([P, S], f16, tag="ss", name="ss")
            dma2(ss_sb, ss_d)
            mk_sb = const.tile([P, P], f16, tag="mk", name="mk")
            nc.sync.dma_start(mk_sb[:], mk_d[:])

            for tq in range(1, NQ):
                load_xq(tq)

            wo_sb = const.tile([P, HQ, HID], f16, tag="wo", name="wo")
            wo_r = wo_d.rearrange("p (h o) -> p h o", h=HQ)
            for h in range(HQ):
                dma2(wo_sb[:, h], wo_r[:, h])

            ident = const.tile([P, P], f16, tag="ident", name="ident")
            make_identity(nc, ident[:])
            ones = const.tile([P, 1], f16, tag="ones", name="ones")
            nc.vector.memset(ones[:], 1.0)

            qR = [const.tile([P, S], f16, tag=f"qR{h}", name=f"qR{h}")
                  for h in range(HQ)]
            kR = const.tile([P, S], f16, tag="kR", name="kR")
            vN = const.tile([P, NKB, D], f16, tag="vN", name="vN")
            avN = [const.tile([P, S], f16, tag=f"avN{h}", name=f"avN{h}")
                   for h in range(HQ)]

            with ExitStack() as ctx:
                # PSUM: proj(2) + s(2) + av(2) + oproj(2) = 8 banks
                proj_pp = ctx.enter_context(
                    tc.tile_pool(name="ps_proj", bufs=2, space="PSUM"))
                s_pp = ctx.enter_context(
                    tc.tile_pool(name="ps_s", bufs=2, space="PSUM"))
                av_pp = ctx.enter_context(
                    tc.tile_pool(name="ps_av", bufs=2, space="PSUM"))
                o_pp = ctx.enter_context(
                    tc.tile_pool(name="ps_o", bufs=2, space="PSUM"))

                def rope_tile(ps, out_sl, tsl):
                    raw = rope_p.tile([P, QT], f16, tag="rp_raw", name="rp_raw")
                    nc.scalar.copy(raw[:], ps[:])
                    sw = rope_p.tile([P, QT], f16, tag="rp_sw", name="rp_sw")
                    nc.sync.dma_start(sw[0:64, :], raw[64:128, :])
                    nc.sync.dma_start(sw[64:128, :], raw[0:64, :])
                    t1 = rope_p.tile([P, QT], f16, tag="rp_t1", name="rp_t1")
                    nc.vector.tensor_mul(t1[:], raw[:], cs_sb[:, tsl])
                    t2 = rope_p.tile([P, QT], f16, tag="rp_t2", name="rp_t2")
                    nc.vector.tensor_mul(t2[:], sw[:], ss_sb[:, tsl])
                    nc.vector.tensor_add(out_sl, t1[:], t2[:])

                def proj_kq0(tq):
                    """k and q-head-0 projections, ko-interleaved so the PE
                    starts on each xT chunk as soon as its DMA lands. Each
                    GEMM accumulates in its own PSUM bank — hardware PSUM
                    accumulation breaks if two start=True groups share a
                    bank."""
                    k_ps = proj_pp.tile([P, QT], f32, tag="proj", name="k_ps")
                    q_ps = proj_pp.tile([P, QT], f32, tag="proj", name="q_ps")
                    for ko in range(KO):
                        st, sp = ko == 0, ko == KO - 1
                        x = xchunk(tq, ko)
                        nc.tensor.matmul(k_ps[:], lhsT=wk_sb[:, ko, :],
                                         rhs=x, start=st, stop=sp)
                        nc.tensor.matmul(q_ps[:], lhsT=wq_sb[:, 0, ko, :],
                                         rhs=x, start=st, stop=sp)
                    tsl = slice(tq * QT, (tq + 1) * QT)
                    rope_tile(k_ps, kR[:, tsl], tsl)
                    rope_tile(q_ps, qR[0][:, tsl], tsl)

                def proj_v(tq):
                    """v projection straight to natural [tok, d] layout:
                    lhsT = xT chunk block, rhs = wv. The four 128-token
                    blocks run sequentially, each in its own psum ring slot
                    (one open accumulation group per bank)."""
                    for i in range(4):
                        v_ps = proj_pp.tile([P, P], f32, tag="proj",
                                            name="v_ps")
                        for ko in range(KO):
                            nc.tensor.matmul(
                                v_ps[:],
                                lhsT=xchunk(tq, ko)[:, i * P:(i + 1) * P],
                                rhs=wv_sb[:, ko, :],
                                start=(ko == 0), stop=(ko == KO - 1))
                        nc.scalar.copy(vN[:, 4 * tq + i, :], v_ps[:])

                def proj_q(tq, h):
                    q_ps = proj_pp.tile([P, QT], f32, tag="proj", name="q_ps")
                    for ko in range(KO):
                        nc.tensor.matmul(
                            q_ps[:], lhsT=wq_sb[:, h, ko, :],
                            rhs=xchunk(tq, ko),
                            start=(ko == 0), stop=(ko == KO - 1))
                    tsl = slice(tq * QT, (tq + 1) * QT)
                    rope_tile(q_ps, qR[h][:, tsl], tsl)

                # ---- attention for query quarter tq, head h ----
                # per-kb: scores -> exp -> (tri-mask) -> {AV matmul, atsum}
                # issue scores[i+1] before AV[i] so the PE always has work
                # while the ScalarE exps the previous block.
                ast = {}

                def attn_scores(tq, h, kb):
                    nkb = 4 * tq + 4
                    q0 = tq * QT
                    if kb == 0:
                        ast[(tq, h)] = {
                            "av": av_pp.tile([P, QT], f32, tag="av", name="av"),
                            "asum": asum_p.tile([P, QT], f16, tag="asum",
                                                name="asum"),
                            "ats": {},
                        }
                    st = ast[(tq, h)]
                    r = kb - 4 * tq
                    lo = P * r if r >= 0 else 0
                    s_ps = s_pp.tile([P, QT], f32, tag="s", name="s")
                    nc.tensor.matmul(
                        s_ps[:, lo:QT],
                        lhsT=kR[:, kb * P:(kb + 1) * P],
                        rhs=qR[h][:, q0 + lo:q0 + QT],
                        start=True, stop=True)
                    at = at_p.tile([P, QT], f16, tag="at", name="at")
                    nc.scalar.activation(at[:, lo:QT], s_ps[:, lo:QT], AF.Exp)
                    if r >= 0:
                        nc.vector.tensor_mul(
                            at[:, lo:lo + P], at[:, lo:lo + P], mk_sb[:])
                    if kb == 0:
                        nc.vector.tensor_copy(st["asum"][:], at[:])
                    else:
                        nc.vector.tensor_add(st["asum"][:, lo:QT],
                                             st["asum"][:, lo:QT], at[:, lo:QT])
                    st["ats"][kb] = at

                def attn_av(tq, h, kb):
                    nkb = 4 * tq + 4
                    q0 = tq * QT
                    st = ast[(tq, h)]
                    r = kb - 4 * tq
                    lo = P * r if r >= 0 else 0
                    at = st["ats"].pop(kb)
                    nc.tensor.matmul(
                        st["av"][:, lo:QT], lhsT=vN[:, kb, :],
                        rhs=at[:, lo:QT],
                        start=(kb == 0), stop=(kb == nkb - 1))
                    if kb == nkb - 1:
                        dn_ps = s_pp.tile([1, QT], f32, tag="s", name="dn_ps")
                        nc.tensor.matmul(dn_ps[0:1, :], lhsT=ones[:],
                                         rhs=st["asum"][:], start=True,
                                         stop=True)
                        rc = small.tile([1, QT], f32, tag="rc", name="rc")
                        nc.vector.reciprocal_approx_fast(rc[:], dn_ps[:])
                        rb = small.tile([P, QT], f32, tag="rb", name="rb")
                        nc.gpsimd.partition_broadcast(rb[:], rc[:])
                        nc.vector.tensor_mul(
                            avN[h][:, q0:q0 + QT], st["av"][:], rb[:])
                        del ast[(tq, h)]

                def attn_quarter(tq):
                    seq = [(h, kb) for h in range(HQ)
                           for kb in range(4 * tq + 4)]
                    attn_scores(tq, seq[0][0], seq[0][1])
                    for i in range(1, len(seq)):
                        attn_scores(tq, *seq[i])
                        attn_av(tq, *seq[i - 1])
                    attn_av(tq, *seq[-1])

                def oproj_quarter(tq):
                    for tb in range(4 * tq, 4 * tq + 4):
                        ot = ost_p.tile([P, HID], f16, tag="ot", name="ot")
                        for ob in range(4):
                            o_ps = o_pp.tile([P, QT], f32, tag="o", name="o_ps")
                            for hh in range(HQ):
                                nc.tensor.matmul(
                                    o_ps[:],
                                    lhsT=avN[hh][:, tb * P:(tb + 1) * P],
                                    rhs=wo_sb[:, hh, ob * QT:(ob + 1) * QT],
                                    start=(hh == 0), stop=(hh == HQ - 1))
                            osl = ot[:, ob * QT:(ob + 1) * QT]
                            if ob % 2 == 0:
                                nc.scalar.copy(osl, o_ps[:])
                            else:
                                nc.vector.tensor_copy(osl, o_ps[:])
                        nc.sync.dma_start(out_r[0:64, tb, :], ot[0:64, :])
                        nc.sync.dma_start(out_r[64:128, tb, :], ot[64:128, :])

                # ---- HAM warmup: keep the PE busy ~3.5us so the clock
                # ramps before the DMA-paced first projections ----
                junk = s_pp.tile([1, QT], f32, tag="s", name="junk")
                for _ in range(32):
                    nc.tensor.matmul(junk[0:1, 0:P], lhsT=ones[:],
                                     rhs=ident[:], start=True, stop=True)

                # ---- streamed schedule ----
                def proj_quarter(tq):
                    proj_kq0(tq)
                    proj_v(tq)
                    for h in range(1, HQ):
                        proj_q(tq, h)

                proj_quarter(0)
                proj_quarter(1)
                attn_quarter(0)
                proj_quarter(2)
                attn_quarter(1)
                oproj_quarter(0)
                proj_quarter(3)
                xt_ctx.close()
                attn_quarter(2)
                oproj_quarter(1)
                attn_quarter(3)
                oproj_quarter(2)
                oproj_quarter(3)

    nc.compile()
    return nc


def _prep_inputs(x, freqs_cis, wq, wk, wv, wo):
    x = np.asarray(x, dtype=np.float32)
    freqs = np.asarray(freqs_cis, dtype=np.float32)
    wq = np.asarray(wq, dtype=np.float32)
    wk = np.asarray(wk, dtype=np.float32)
    wv = np.asarray(wv, dtype=np.float32)
    wo = np.asarray(wo, dtype=np.float32)

    perm = np.concatenate([np.arange(0, D, 2), np.arange(1, D, 2)])
    cos = freqs[..., 0].T.astype(np.float32)            # [64, S]
    sin = freqs[..., 1].T.astype(np.float32)
    cs2 = np.ascontiguousarray(np.concatenate([cos, cos], 0)).astype(F16)
    ss2 = np.ascontiguousarray(np.concatenate([-sin, sin], 0)).astype(F16)

    wq_p = (wq.reshape(HID, H, D)[:, :, perm] * D**-0.5).astype(F16)
    wk_p = wk.reshape(HID, KVH, D)[:, :, perm].astype(F16)
    wv_r = wv.reshape(HID, KVH, D).astype(F16)
    wo_r = wo.reshape(H, D, HID)

    kk = np.arange(P)[:, None]
    qq = np.arange(P)[None, :]
    tri = (kk <= qq).astype(F16)                        # [128, 128]

    # xT chunk groups: row (tq, kg, p) holds 4 ko-chunks of 512 tokens each
    # (4KB contiguous per partition -> one DMA descriptor per partition)
    xT = x.transpose(0, 2, 1).reshape(B, 4, 4, P, NQ, QT)  # [b,kg,j,p,tq,qt]
    xT = np.ascontiguousarray(xT.transpose(0, 4, 1, 3, 2, 5)).astype(F16)
    xT = xT.reshape(B, NQ * 4 * P, 4 * QT)

    def swz(w):  # [HID, N] -> [P, KO*N] so each partition's DMA is contiguous
        n = w.shape[1]
        return np.ascontiguousarray(
            w.reshape(KO, P, n).transpose(1, 0, 2).reshape(P, KO * n))

    in_maps = []
    for c in range(N_CORES):
        b, g = c // 4, c % 4
        # wq host layout [P, HQ, KO, D]: per-head contiguous for split DMAs
        wq_g = wq_p[:, 4 * g:4 * g + HQ, :]             # [HID, HQ, D]
        wq_sw = wq_g.reshape(KO, P, HQ, D).transpose(1, 2, 0, 3)
        wq_sw = np.ascontiguousarray(wq_sw).reshape(P, HQ * KO * D)
        wo_g = wo_r[4 * g:4 * g + HQ].astype(F16)       # [HQ, P, HID]
        in_maps.append({
            "xT": xT[b],
            "wq": wq_sw,
            "wk": swz(wk_p[:, g, :]),
            "wv": swz(wv_r[:, g, :]),
            "wo": np.ascontiguousarray(
                wo_g.transpose(1, 0, 2).reshape(P, HQ * HID)),
            "cs2": cs2,
            "ss2": ss2,
            "mask": tri,
        })
    return in_maps


def _ensure_ntff_hook():
    """Optional: register the NTFF profiling hook if the image's antenv lacks
    it, so BASS_TRACE=1 produces a profile instead of crashing. No-op on
    failure or when the hook already exists."""
    import sys as _sys
    import types as _types
    try:
        from antenv.axon_hooks import get_axon_ntff_profile_hook  # noqa: F401
        return
    except ImportError:
        pass
    try:
        from trn_agent_boot.trn_boot import _ntff_profile_via_ctypes
        hook = _ntff_profile_via_ctypes("/opt/axon/libaxon_pjrt.so")
        mod = _types.ModuleType("antenv.axon_hooks")
        mod.get_axon_ntff_profile_hook = lambda: hook
        mod.set_axon_ntff_profile_hook = lambda h: None
        _sys.modules["antenv.axon_hooks"] = mod
    except Exception:
        pass


def kernel(x, freqs_cis, wq, wk, wv, wo):
    from concourse.bass_utils import run_bass_kernel_spmd
    _ensure_ntff_hook()

    nc = _CACHE.get("nc")
    if nc is None:
        nc = _build_nc()
        _CACHE["nc"] = nc

    in_maps = _prep_inputs(x, freqs_cis, wq, wk, wv, wo)
    res = run_bass_kernel_spmd(nc, in_maps, list(range(N_CORES)))
    _CACHE["last_result"] = res
    parts = [np.asarray(res.results[c]["out"]).astype(np.float32)
             for c in range(N_CORES)]
    out = np.stack([parts[0] + parts[1] + parts[2] + parts[3],
                    parts[4] + parts[5] + parts[6] + parts[7]])
    return out


# revision 18
# speedup vs baseline: 1.1415x; 1.1089x over previous
"""Causal GQA attention (B=2, S=2048, HID=2048, H=16, KVH=4, D=128) on 8 TRN2 cores.

Sharding: core c -> batch c//4, kv-group c%4 (4 q-heads + 1 kv-head).
o_proj is row-split by head group; host sums the 4 partials per batch.

Device kernel (per core, fp16 matmuls / f32 accumulation), fully streamed:
the sequence is processed in four 512-token quarters. Per quarter: v/k
projections (ko-interleaved so PE starts as soon as each xT chunk lands),
q projections, then attention for that query quarter (kv blocks 0..end of
quarter) and o_proj for the previous quarter are interleaved between later
projections so the PE never sits behind a DMA or phase barrier.

Softmax denominators are NOT computed with ones-matmuls on the PE (that
doubles attention PE traffic); instead the exp tiles are accumulated
elementwise on the DVE (fp16 atsum) and a single [1,512] ones-matmul per
(head, quarter) reduces across partitions.
"""

import numpy as np
import ml_dtypes

F16 = np.float16

B, S, HID = 2, 2048, 2048
H, KVH, D = 16, 4, 128
P = 128
KO = HID // P          # 16 contraction chunks of 128
HQ = H // KVH          # 4 q heads per core
NQ = 4                 # token quarters
QT = S // NQ           # 512 tokens per quarter
NKB = S // P           # 16 key blocks of 128
N_CORES = 8

_CACHE = {}


def _build_nc():
    import concourse.tile as tile
    from concourse import bacc, mybir
    from concourse.masks import make_identity
    from contextlib import ExitStack

    f16 = mybir.dt.float16
    f32 = mybir.dt.float32
    AF = mybir.ActivationFunctionType

    nc = bacc.Bacc("TRN2", target_bir_lowering=False, debug=False,
                   num_devices=N_CORES)

    xT_d = nc.dram_tensor("xT", [NQ * 4 * P, 4 * QT], f16, kind="ExternalInput").ap()
    wq_d = nc.dram_tensor("wq", [P, HQ * KO * D], f16, kind="ExternalInput").ap()
    wk_d = nc.dram_tensor("wk", [P, KO * D], f16, kind="ExternalInput").ap()
    wv_d = nc.dram_tensor("wv", [P, KO * D], f16, kind="ExternalInput").ap()
    wo_d = nc.dram_tensor("wo", [P, HQ * HID], f16, kind="ExternalInput").ap()
    cs_d = nc.dram_tensor("cs2", [P, S], f16, kind="ExternalInput").ap()
    ss_d = nc.dram_tensor("ss2", [P, S], f16, kind="ExternalInput").ap()
    mk_d = nc.dram_tensor("mask", [P, P], f16, kind="ExternalInput").ap()
    out_d = nc.dram_tensor("out", [S, HID], f16, kind="ExternalOutput").ap()
    out_r = out_d.rearrange("(tb p) o -> p tb o", p=P)

    with tile.TileContext(nc) as tc:
        with ExitStack() as octx:
            const = octx.enter_context(tc.tile_pool(name="const", bufs=1))
            rope_p = octx.enter_context(tc.tile_pool(name="rope", bufs=3))
            at_p = octx.enter_context(tc.tile_pool(name="at", bufs=5))
            asum_p = octx.enter_context(tc.tile_pool(name="asum", bufs=2))
            small = octx.enter_context(tc.tile_pool(name="small", bufs=3))
            ost_p = octx.enter_context(tc.tile_pool(name="ost", bufs=4))
            xt_ctx = ExitStack()
            xt_pool = xt_ctx.enter_context(tc.tile_pool(name="xt", bufs=1))

            # ---- persistent input DMAs, emitted in consumption order.
            # The DMA queues are descriptor-rate-bound (~4KB/partition per
            # descriptor is the sweet spot) and one dma_start lands on one
            # queue, so every big tensor is packed into 4KB-per-partition
            # runs and split into partition-halves to parallelize across
            # queues. ----
            def dmaN(dst, src, n):
                step = P // n
                for i in range(n):
                    nc.sync.dma_start(dst[i * step:(i + 1) * step],
                                      src[i * step:(i + 1) * step])

            def dma2(dst, src):
                dmaN(dst, src, 2)

            # first-needed tensors in partition-quarters (4 queues each)
            wk_sb = const.tile([P, KO, D], f16, tag="wk", name="wk")
            dmaN(wk_sb, wk_d.rearrange("p (ko n) -> p ko n", ko=KO), 4)
            wv_sb = const.tile([P, KO, D], f16, tag="wv", name="wv")
            dmaN(wv_sb, wv_d.rearrange("p (ko n) -> p ko n", ko=KO), 4)

            wq_sb = const.tile([P, HQ, KO, D], f16, tag="wq", name="wq")
            wq_r = wq_d.rearrange("p (h ko n) -> p h ko n", h=HQ, ko=KO)
            dmaN(wq_sb[:, 0], wq_r[:, 0], 4)

            # xT arrives as groups of 4 ko-chunks (4KB/partition per DMA)
            xTg = [[None] * 4 for _ in range(NQ)]

            def load_xq(tq, n=2):
                for kg in range(4):
                    t = xt_pool.tile([P, 4, QT], f16, tag=f"xT{tq}_{kg}",
                                     name=f"xT{tq}_{kg}")
                    base = (tq * 4 + kg) * P
                    dmaN(t.rearrange("p a b -> p (a b)"),
                         xT_d[base:base + P, :], n)
                    xTg[tq][kg] = t

            def xchunk(tq, ko):
                return xTg[tq][ko // 4][:, ko % 4, :]

            load_xq(0, n=4)

            for h in range(1, HQ):
                dma2(wq_sb[:, h], wq_r[:, h])

            cs_sb = const.tile([P, S], f16, tag="cs", name="cs")
            dma2(cs_sb, cs_d)
            ss_sb = const.tile([P, S], f16, tag="ss", name="ss")
            dma2(ss_sb, ss_d)
            mk_sb = const.tile([P, P], f16, tag="mk", name="mk")
            nc.sync.dma_start(mk_sb[:], mk_d[:])

            for tq in range(1, NQ):
                load_xq(tq)

            wo_sb = const.tile([P, HQ, HID], f16, tag="wo", name="wo")
            wo_r = wo_d.rearrange("p (h o) -> p h o", h=HQ)
            for h in range(HQ):
                dma2(wo_sb[:, h], wo_r[:, h])

            ident = const.tile([P, P], f16, tag="ident", name="ident")
            make_identity(nc, ident[:])
            ones = const.tile([P, 1], f16, tag="ones", name="ones")
            nc.vector.memset(ones[:], 1.0)

            qR = [const.tile([P, S], f16, tag=f"qR{h}", name=f"qR{h}")
                  for h in range(HQ)]
            kR = const.tile([P, S], f16, tag="kR", name="kR")
            vN = const.tile([P, NKB, D], f16, tag="vN", name="vN")
            avN = [const.tile([P, S], f16, tag=f"avN{h}", name=f"avN{h}")
                   for h in range(HQ)]

            with ExitStack() as ctx:
                # PSUM: proj(2) + s(2) + av(2) + oproj(2) = 8 banks
                proj_pp = ctx.enter_context(
                    tc.tile_pool(name="ps_proj", bufs=2, space="PSUM"))
                s_pp = ctx.enter_context(
                    tc.tile_pool(name="ps_s", bufs=2, space="PSUM"))
                av_pp = ctx.enter_context(
                    tc.tile_pool(name="ps_av", bufs=2, space="PSUM"))
                o_pp = ctx.enter_context(
                    tc.tile_pool(name="ps_o", bufs=2, space="PSUM"))

                # rotate partitions by 64 (stream_shuffle moves 4-partition
                # groups); keeps the rope half-swap off the DMA queues
                SWAP64 = list(range(16, 32)) + list(range(16))

                def rope_tile(ps, out_sl, tsl):
                    raw = rope_p.tile([P, QT], f16, tag="rp_raw", name="rp_raw")
                    nc.scalar.copy(raw[:], ps[:])
                    sw = rope_p.tile([P, QT], f16, tag="rp_sw", name="rp_sw")
                    nc.vector.stream_shuffle(sw[:], raw[:], SWAP64)
                    t1 = rope_p.tile([P, QT], f16, tag="rp_t1", name="rp_t1")
                    nc.vector.tensor_mul(t1[:], raw[:], cs_sb[:, tsl])
                    t2 = rope_p.tile([P, QT], f16, tag="rp_t2", name="rp_t2")
                    nc.vector.tensor_mul(t2[:], sw[:], ss_sb[:, tsl])
                    nc.vector.tensor_add(out_sl, t1[:], t2[:])

                def proj_kq0(tq):
                    """k and q-head-0 projections, ko-interleaved so the PE
                    starts on each xT chunk as soon as its DMA lands. Each
                    GEMM accumulates in its own PSUM bank — hardware PSUM
                    accumulation breaks if two start=True groups share a
                    bank."""
                    k_ps = proj_pp.tile([P, QT], f32, tag="proj", name="k_ps")
                    q_ps = proj_pp.tile([P, QT], f32, tag="proj", name="q_ps")
                    for ko in range(KO):
                        st, sp = ko == 0, ko == KO - 1
                        x = xchunk(tq, ko)
                        nc.tensor.matmul(k_ps[:], lhsT=wk_sb[:, ko, :],
                                         rhs=x, start=st, stop=sp)
                        nc.tensor.matmul(q_ps[:], lhsT=wq_sb[:, 0, ko, :],
                                         rhs=x, start=st, stop=sp)
                    tsl = slice(tq * QT, (tq + 1) * QT)
                    rope_tile(k_ps, kR[:, tsl], tsl)
                    rope_tile(q_ps, qR[0][:, tsl], tsl)

                def proj_v(tq):
                    """v projection straight to natural [tok, d] layout:
                    lhsT = xT chunk block, rhs = wv. The four 128-token
                    blocks run sequentially, each in its own psum ring slot
                    (one open accumulation group per bank)."""
                    for i in range(4):
                        v_ps = proj_pp.tile([P, P], f32, tag="proj",
                                            name="v_ps")
                        for ko in range(KO):
                            nc.tensor.matmul(
                                v_ps[:],
                                lhsT=xchunk(tq, ko)[:, i * P:(i + 1) * P],
                                rhs=wv_sb[:, ko, :],
                                start=(ko == 0), stop=(ko == KO - 1))
                        nc.scalar.copy(vN[:, 4 * tq + i, :], v_ps[:])

                def proj_q(tq, h):
                    q_ps = proj_pp.tile([P, QT], f32, tag="proj", name="q_ps")
                    for ko in range(KO):
                        nc.tensor.matmul(
                            q_ps[:], lhsT=wq_sb[:, h, ko, :],
                            rhs=xchunk(tq, ko),
                            start=(ko == 0), stop=(ko == KO - 1))
                    tsl = slice(tq * QT, (tq + 1) * QT)
                    rope_tile(q_ps, qR[h][:, tsl], tsl)

                # ---- attention for query quarter tq, head h ----
                # per-kb: scores -> exp -> (tri-mask) -> {AV matmul, atsum}
                # issue scores[i+1] before AV[i] so the PE always has work
                # while the ScalarE exps the previous block.
                ast = {}

                def attn_scores(tq, h, kb):
                    nkb = 4 * tq + 4
                    q0 = tq * QT
                    if kb == 0:
                        ast[(tq, h)] = {
                            "av": av_pp.tile([P, QT], f32, tag="av", name="av"),
                            "asum": asum_p.tile([P, QT], f16, tag="asum",
                                                name="asum"),
                            "ats": {},
                        }
                    st = ast[(tq, h)]
                    r = kb - 4 * tq
                    lo = P * r if r >= 0 else 0
                    s_ps = s_pp.tile([P, QT], f32, tag="s", name="s")
                    nc.tensor.matmul(
                        s_ps[:, lo:QT],
                        lhsT=kR[:, kb * P:(kb + 1) * P],
                        rhs=qR[h][:, q0 + lo:q0 + QT],
                        start=True, stop=True)
                    at = at_p.tile([P, QT], f16, tag="at", name="at")
                    nc.scalar.activation(at[:, lo:QT], s_ps[:, lo:QT], AF.Exp)
                    if r >= 0:
                        nc.vector.tensor_mul(
                            at[:, lo:lo + P], at[:, lo:lo + P], mk_sb[:])
                    if kb == 0:
                        nc.vector.tensor_copy(st["asum"][:], at[:])
                    else:
                        nc.vector.tensor_add(st["asum"][:, lo:QT],
                                             st["asum"][:, lo:QT], at[:, lo:QT])
                    st["ats"][kb] = at

                def attn_av(tq, h, kb):
                    nkb = 4 * tq + 4
                    q0 = tq * QT
                    st = ast[(tq, h)]
                    r = kb - 4 * tq
                    lo = P * r if r >= 0 else 0
                    at = st["ats"].pop(kb)
                    nc.tensor.matmul(
                        st["av"][:, lo:QT], lhsT=vN[:, kb, :],
                        rhs=at[:, lo:QT],
                        start=(kb == 0), stop=(kb == nkb - 1))
                    if kb == nkb - 1:
                        dn_ps = s_pp.tile([1, QT], f32, tag="s", name="dn_ps")
                        nc.tensor.matmul(dn_ps[0:1, :], lhsT=ones[:],
                                         rhs=st["asum"][:], start=True,
                                         stop=True)
                        rc = small.tile([1, QT], f32, tag="rc", name="rc")
                        nc.vector.reciprocal_approx_fast(rc[:], dn_ps[:])
                        rb = small.tile([P, QT], f32, tag="rb", name="rb")
                        nc.gpsimd.partition_broadcast(rb[:], rc[:])
                        nc.vector.tensor_mul(
                            avN[h][:, q0:q0 + QT], st["av"][:], rb[:])
                        del ast[(tq, h)]

                def attn_quarter(tq):
                    seq = [(h, kb) for h in range(HQ)
                           for kb in range(4 * tq + 4)]
                    attn_scores(tq, seq[0][0], seq[0][1])
                    for i in range(1, len(seq)):
                        attn_scores(tq, *seq[i])
                        attn_av(tq, *seq[i - 1])
                    attn_av(tq, *seq[-1])

                def oproj_quarter(tq):
                    for tb in range(4 * tq, 4 * tq + 4):
                        ot = ost_p.tile([P, HID], f16, tag="ot", name="ot")
                        for ob in range(4):
                            o_ps = o_pp.tile([P, QT], f32, tag="o", name="o_ps")
                            for hh in range(HQ):
                                nc.tensor.matmul(
                                    o_ps[:],
                                    lhsT=avN[hh][:, tb * P:(tb + 1) * P],
                                    rhs=wo_sb[:, hh, ob * QT:(ob + 1) * QT],
                                    start=(hh == 0), stop=(hh == HQ - 1))
                            osl = ot[:, ob * QT:(ob + 1) * QT]
                            if ob % 2 == 0:
                                nc.scalar.copy(osl, o_ps[:])
                            else:
                                nc.vector.tensor_copy(osl, o_ps[:])
                        for i in range(4):
                            nc.sync.dma_start(out_r[i * 32:(i + 1) * 32, tb, :],
                                              ot[i * 32:(i + 1) * 32, :])

                # ---- HAM warmup: keep the PE busy ~3.5us so the clock
                # ramps before the DMA-paced first projections ----
                junk = s_pp.tile([1, QT], f32, tag="s", name="junk")
                for _ in range(40):
                    nc.tensor.matmul(junk[0:1, 0:P], lhsT=ones[:],
                                     rhs=ident[:], start=True, stop=True)

                # ---- streamed schedule ----
                def proj_quarter(tq):
                    proj_kq0(tq)
                    proj_v(tq)
                    for h in range(1, HQ):
                        proj_q(tq, h)

                proj_quarter(0)
                attn_quarter(0)
                proj_quarter(1)
                attn_quarter(1)
                proj_quarter(2)
                oproj_quarter(0)
                proj_quarter(3)
                xt_ctx.close()
                attn_quarter(2)
                oproj_quarter(1)
                attn_quarter(3)
                oproj_quarter(2)
                oproj_quarter(3)

    nc.compile()
    return nc


def _prep_inputs(x, freqs_cis, wq, wk, wv, wo):
    x = np.asarray(x, dtype=np.float32)
    freqs = np.asarray(freqs_cis, dtype=np.float32)
    wq = np.asarray(wq, dtype=np.float32)
    wk = np.asarray(wk, dtype=np.float32)
    wv = np.asarray(wv, dtype=np.float32)
    wo = np.asarray(wo, dtype=np.float32)

    # RoPE row layout: partition 32*qd + i (i<16) = real part of pair
    # 16*qd + i, partition 32*qd + 16 + i = its imag part. Partners sit in
    # the same 32-partition quadrant so DVE stream_shuffle (quadrant-local)
    # can do the half-swap without a DMA.
    pairidx = np.empty(P, dtype=np.int64)
    sign = np.empty(P, dtype=np.float32)
    perm = np.empty(P, dtype=np.int64)
    for qd in range(4):
        for i in range(16):
            pairidx[32 * qd + i] = 16 * qd + i
            pairidx[32 * qd + 16 + i] = 16 * qd + i
            sign[32 * qd + i] = -1.0
            sign[32 * qd + 16 + i] = 1.0
            perm[32 * qd + i] = 2 * (16 * qd + i)
            perm[32 * qd + 16 + i] = 2 * (16 * qd + i) + 1
    cos = freqs[..., 0].T.astype(np.float32)            # [64, S]
    sin = freqs[..., 1].T.astype(np.float32)
    cs2 = np.ascontiguousarray(cos[pairidx, :]).astype(F16)
    ss2 = np.ascontiguousarray(sign[:, None] * sin[pairidx, :]).astype(F16)

    wq_p = (wq.reshape(HID, H, D)[:, :, perm] * D**-0.5).astype(F16)
    wk_p = wk.reshape(HID, KVH, D)[:, :, perm].astype(F16)
    wv_r = wv.reshape(HID, KVH, D).astype(F16)
    wo_r = wo.reshape(H, D, HID)

    kk = np.arange(P)[:, None]
    qq = np.arange(P)[None, :]
    tri = (kk <= qq).astype(F16)                        # [128, 128]

    # xT chunk groups: row (tq, kg, p) holds 4 ko-chunks of 512 tokens each
    # (4KB contiguous per partition -> one DMA descriptor per partition)
    xT = x.transpose(0, 2, 1).reshape(B, 4, 4, P, NQ, QT)  # [b,kg,j,p,tq,qt]
    xT = np.ascontiguousarray(xT.transpose(0, 4, 1, 3, 2, 5)).astype(F16)
    xT = xT.reshape(B, NQ * 4 * P, 4 * QT)

    def swz(w):  # [HID, N] -> [P, KO*N] so each partition's DMA is contiguous
        n = w.shape[1]
        return np.ascontiguousarray(
            w.reshape(KO, P, n).transpose(1, 0, 2).reshape(P, KO * n))

    in_maps = []
    for c in range(N_CORES):
        b, g = c // 4, c % 4
        # wq host layout [P, HQ, KO, D]: per-head contiguous for split DMAs
        wq_g = wq_p[:, 4 * g:4 * g + HQ, :]             # [HID, HQ, D]
        wq_sw = wq_g.reshape(KO, P, HQ, D).transpose(1, 2, 0, 3)
        wq_sw = np.ascontiguousarray(wq_sw).reshape(P, HQ * KO * D)
        wo_g = wo_r[4 * g:4 * g + HQ].astype(F16)       # [HQ, P, HID]
        in_maps.append({
            "xT": xT[b],
            "wq": wq_sw,
            "wk": swz(wk_p[:, g, :]),
            "wv": swz(wv_r[:, g, :]),
            "wo": np.ascontiguousarray(
                wo_g.transpose(1, 0, 2).reshape(P, HQ * HID)),
            "cs2": cs2,
            "ss2": ss2,
            "mask": tri,
        })
    return in_maps


def _ensure_ntff_hook():
    """Optional: register the NTFF profiling hook if the image's antenv lacks
    it, so BASS_TRACE=1 produces a profile instead of crashing. No-op on
    failure or when the hook already exists."""
    import sys as _sys
    import types as _types
    try:
        from antenv.axon_hooks import get_axon_ntff_profile_hook  # noqa: F401
        return
    except ImportError:
        pass
    try:
        from trn_agent_boot.trn_boot import _ntff_profile_via_ctypes
        hook = _ntff_profile_via_ctypes("/opt/axon/libaxon_pjrt.so")
        mod = _types.ModuleType("antenv.axon_hooks")
        mod.get_axon_ntff_profile_hook = lambda: hook
        mod.set_axon_ntff_profile_hook = lambda h: None
        _sys.modules["antenv.axon_hooks"] = mod
    except Exception:
        pass


def kernel(x, freqs_cis, wq, wk, wv, wo):
    from concourse.bass_utils import run_bass_kernel_spmd
    _ensure_ntff_hook()

    nc = _CACHE.get("nc")
    if nc is None:
        nc = _build_nc()
        _CACHE["nc"] = nc

    in_maps = _prep_inputs(x, freqs_cis, wq, wk, wv, wo)
    res = run_bass_kernel_spmd(nc, in_maps, list(range(N_CORES)))
    _CACHE["last_result"] = res
    parts = [np.asarray(res.results[c]["out"]).astype(np.float32)
             for c in range(N_CORES)]
    out = np.stack([parts[0] + parts[1] + parts[2] + parts[3],
                    parts[4] + parts[5] + parts[6] + parts[7]])
    return out
